# revision 37
# baseline (speedup 1.0000x reference)
"""BERT_BiLSTM_CRF loss (CRF NLL) Trainium2 kernel, v2.

Forward-backward meet-in-the-middle CRF forward scores + PE-based gold scores.

Per core (BL=128 seqs on partitions):
  - alpha chain (t=0..1023) and beta chain (q=0..1024) run as ONE dual-batched
    exp-space recurrence on DVE: state [BL,2,7], step matrices Mab [BL,2,7,7]
    built in bulk on Pool from ACT exp(feats-G). beta runs on host-reversed
    per-length zero-padded feats. fwd = ln(alpha_m . beta_q) + offsets for
    long seqs, ln(E8 . alpha_{len-1}) + offsets for short.
  - Renorm by running max every RN dual-steps (in-place on the stored history
    slot, per lane); ln(max) batched on ACT; prefix-sums via DVE scan.
  - History SBUF-resident, one flush to DRAM, per-seq indirect-DMA gathers
    with host-precomputed indices.
  - Gold via per-seq PE matmuls on host-transposed one-hot/feat planes:
    PSUM[j, s, 0:8]  = sum_t ohm[t,j] ohprev8[t,i]  (pair counts, START col)
    PSUM[j, s, 8:15] = sum_t ohm[t,j] featT[t,j']   (diagonal = feat part)
    dotted against [trans[0:7,0:8] | I7]; plus the STOP-row last-tag term.
Output: per-seq (fwd - gold) [BL,1] f32; host takes the mean.
"""

import os

import numpy as np

# Reset NeuronCores at runtime init: recovers degraded clock/p-state left by
# earlier runs (measured: identical NEFF 835us fresh vs 1000us degraded).
os.environ.setdefault("NEURON_RT_RESET_CORES", "1")

B, T, K = 1024, 2048, 9
NCORES = 8
BL = B // NCORES
KA = 7
START, STOP = 7, 8
G = 2.4
H = T // 2                 # chain length (dual steps)
RN = 32                    # dual-steps between renorms
NEV = H // RN              # renorm events per lane
CT = 128                   # dual-steps per M-build chunk
NCH = H // CT
TC_ = 128                  # t per transposed gold chunk
NGC = T // TC_             # gold chunks
NPS = 15                   # psum cols per seq (8 pair + 7 feat)

_CACHE = {}
TRACE = False


def _build_bass():
    import contextlib

    import concourse.bass as bass
    import concourse.bacc as bacc
    import concourse.mybir as mybir
    import concourse.tile as tile

    f32 = mybir.dt.float32
    bf16 = mybir.dt.bfloat16
    i32 = mybir.dt.int32
    AX = mybir.AxisListType
    OP = mybir.AluOpType
    AF = mybir.ActivationFunctionType

    nc = bacc.Bacc()

    # ---- DRAM inputs ----
    fwd_f = nc.dram_tensor("fwd_f", [BL, H, KA], bf16, kind="ExternalInput")
    rev_f = nc.dram_tensor("rev_f", [BL, H, KA], bf16, kind="ExternalInput")
    featT = nc.dram_tensor("featT", [TC_, NGC, BL, KA], bf16, kind="ExternalInput")
    tagT = nc.dram_tensor("tagT", [TC_, NGC, BL], bf16, kind="ExternalInput")
    ohmT = nc.dram_tensor("ohmT", [TC_, NGC, BL, KA], bf16, kind="ExternalInput")
    ohpT = nc.dram_tensor("ohpT", [TC_, NGC, BL, KA + 1], bf16,
                          kind="ExternalInput")
    lenf = nc.dram_tensor("lenf", [BL, 1], f32, kind="ExternalInput")
    hidx = nc.dram_tensor("hidx", [BL, 1], i32, kind="ExternalInput")
    oidx = nc.dram_tensor("oidx", [BL, 1], i32, kind="ExternalInput")
    lidx = nc.dram_tensor("lidx", [BL, 1], i32, kind="ExternalInput")
    islf = nc.dram_tensor("islf", [BL, 1], f32, kind="ExternalInput")
    trans = nc.dram_tensor("trans", [K, K], f32, kind="ExternalInput")
    patR = nc.dram_tensor("patR", [KA, NPS], f32, kind="ExternalInput")
    outv = nc.dram_tensor("outv", [BL, 1], f32, kind="ExternalOutput")

    # DRAM scratch
    dh_d = nc.dram_tensor("dh_d", [BL * (H + 1) * 2, KA], bf16)
    off_d = nc.dram_tensor("off_d", [2 * BL, NEV + 1], f32)
    gvec_d = nc.dram_tensor("gvec_d", [KA, BL], f32)

    iota7_np = np.arange(KA, dtype=np.float32).reshape(1, KA)
    c_iota7 = nc.inline_tensor(iota7_np, "c_iota7")
    iota8p1_np = np.arange(1, KA + 2, dtype=np.float32).reshape(1, KA + 1)
    c_iota8p1 = nc.inline_tensor(iota8p1_np, "c_iota8p1")

    with tile.TileContext(nc) as tc:
        ctx = contextlib.ExitStack()
        with ctx, nc.allow_low_precision(reason="bf16 CRF state, validated"):
            sing = ctx.enter_context(tc.tile_pool(name="sing", bufs=1))
            fpool = ctx.enter_context(tc.tile_pool(name="fpool", bufs=2))
            gpool = ctx.enter_context(tc.tile_pool(name="gpool", bufs=2))
            spool = ctx.enter_context(tc.tile_pool(name="spool", bufs=1))
            psum = ctx.enter_context(tc.tile_pool(name="psum", bufs=2,
                                                  space="PSUM"))

            # ---------- constants ----------
            transb = sing.tile([BL, K * K], f32)
            nc.sync.dma_start(transb[:], bass.AP(trans, 0, [[0, BL], [1, K * K]]))
            trv = transb[:].rearrange("p (j i) -> p j i", i=K)
            lenf_sb = sing.tile([BL, 1], f32)
            nc.sync.dma_start(lenf_sb[:], lenf[:, :])
            iota7 = sing.tile([BL, KA], f32)
            nc.sync.dma_start(iota7[:], bass.AP(c_iota7, 0, [[0, BL], [1, KA]]))
            hidx_sb = sing.tile([BL, 1], i32)
            nc.sync.dma_start(hidx_sb[:], hidx[:, :])
            oidx_sb = sing.tile([BL, 1], i32)
            nc.sync.dma_start(oidx_sb[:], oidx[:, :])
            lidx_sb = sing.tile([BL, 1], i32)
            nc.sync.dma_start(lidx_sb[:], lidx[:, :])
            isl = sing.tile([BL, 1], f32)
            nc.sync.dma_start(isl[:], islf[:, :])

            # E2[s, 0, j, i] = exp(trans[j, i]); E2[s, 1, a, b] = exp(trans[b, a])
            E2 = sing.tile([BL, 2, KA, KA], bf16)
            nc.scalar.activation(E2[:, 0, :, :], trv[:, 0:KA, 0:KA], AF.Exp)
            nc.scalar.activation(
                E2[:, 1, :, :],
                trv[:, 0:KA, 0:KA].rearrange("p j i -> p i j"), AF.Exp)
            E7E = sing.tile([BL, KA], bf16)      # exp(trans[j, START])
            nc.scalar.activation(E7E[:], trv[:, 0:KA, START:START + 1], AF.Exp)
            E8E = sing.tile([BL, KA], f32)       # exp(trans[STOP, j])
            nc.scalar.activation(E8E[:], trv[:, STOP:STOP + 1, 0:KA], AF.Exp)
            negG = sing.tile([BL, 1], f32)
            nc.vector.memset(negG[:], -G)

            # ---------- emissions (chunk-transient): efc local slot x holds
            # lane0 = exp(fwd[c*CT+x]-G) and lane1 = exp(rev[c*CT+x-1]-G),
            # x = 1..CT, exactly what M-build chunk c consumes ---------------
            ef00 = sing.tile([BL, 1, 2, KA], bf16)
            f0t = fpool.tile([BL, 1, KA], bf16, tag="fa")
            nc.sync.dma_start(f0t[:], fwd_f[:, 0:1, :])
            nc.scalar.activation(ef00[:, :, 0, :], f0t[:], AF.Exp,
                                 bias=negG[:, 0:1])
            f0r = fpool.tile([BL, 1, KA], bf16, tag="fb")
            nc.sync.dma_start(f0r[:], rev_f[:, 0:1, :])
            nc.scalar.activation(ef00[:, :, 1, :], f0r[:], AF.Exp,
                                 bias=negG[:, 0:1])

            # ---------- dual chain ----------
            Dhist = sing.tile([BL, H + 1, 2, KA], bf16)

            nc.vector.tensor_tensor(
                Dhist[:, 0, 0, :], E7E[:], ef00[:, 0, 0, :], op=OP.mult)
            nc.vector.tensor_tensor(
                Dhist[:, 0, 1, :], E8E[:], ef00[:, 0, 1, :], op=OP.mult)

            rcd = sing.tile([BL, 1], f32)
            MXs = sing.tile([BL, NEV], f32)
            dums = sing.tile([BL, NCH], f32)
            MT = [sing.tile([BL, CT, 2, KA, KA], bf16, name=f"MT{i}")
                  for i in range(5)]
            pend_at = [-1]
            for c in range(NCH):
                # fwd rows c*CT+1 .. c*CT+CT (last chunk: 127 rows + pad=1.0)
                nfw = CT if c < NCH - 1 else CT - 1
                efc = fpool.tile([BL, CT, 2, KA], bf16, tag="efc")
                fa = fpool.tile([BL, CT, KA], bf16, tag="fa")
                nc.sync.dma_start(fa[:, 0:nfw, :],
                                  fwd_f[:, c * CT + 1:c * CT + 1 + nfw, :])
                nc.scalar.activation(
                    efc[:, 0:nfw, 0, :], fa[:, 0:nfw, :],
                    AF.Exp, bias=negG[:, 0:1])
                if c == NCH - 1:
                    nc.vector.memset(efc[:, CT - 1, 0, :], 1.0)
                fb = fpool.tile([BL, CT, KA], bf16, tag="fb")
                nc.sync.dma_start(fb[:], rev_f[:, c * CT:(c + 1) * CT, :])
                nc.scalar.activation(
                    efc[:, :, 1, :], fb[:], AF.Exp, bias=negG[:, 0:1])

                Mab = MT[c % 5]
                ka0 = c * CT + 1
                nc.gpsimd.tensor_tensor(
                    Mab[:],
                    E2[:].unsqueeze(1).broadcast_to([BL, CT, 2, KA, KA]),
                    efc[:].unsqueeze(4).broadcast_to([BL, CT, 2, KA, KA]),
                    op=OP.mult)

                Xab = spool.tile([BL, 2, KA, KA], bf16, tag="Xab")
                for l in range(CT):
                    k = c * CT + l + 1
                    sprev = (Dhist[:, k - 1, :, :].unsqueeze(2)
                             .broadcast_to([BL, 2, KA, KA]))
                    if k == pend_at[0]:
                        for ch in range(2):
                            nc.vector.scalar_tensor_tensor(
                                out=Xab[:, ch, :, :],
                                in0=Mab[:, l, ch, :, :],
                                scalar=rcd[:, 0:1],
                                in1=Dhist[:, k - 1, ch, :].unsqueeze(1)
                                    .broadcast_to([BL, KA, KA]),
                                op0=OP.mult, op1=OP.mult)
                        pend_at[0] = -1
                    else:
                        nc.vector.tensor_tensor(
                            Xab[:], Mab[:, l, :, :, :], sprev, op=OP.mult)
                    nc.vector.tensor_reduce(
                        out=Dhist[:, k, :, :], in_=Xab[:], axis=AX.X, op=OP.add)
                    if k % RN == 0 and k < H:
                        ev = k // RN - 1
                        nc.vector.tensor_reduce(
                            out=MXs[:, ev:ev + 1], in_=Dhist[:, k, :, :],
                            axis=AX.XY, op=OP.max)
                        nc.vector.reciprocal(rcd[:], MXs[:, ev:ev + 1])
                        pend_at[0] = k + 4
                k0 = c * CT + 1 if c > 0 else 0
                k1 = (c + 1) * CT + 1
                nc.sync.dma_start(
                    bass.AP(dh_d, k0 * 2 * KA,
                            [[(H + 1) * 2 * KA, BL], [1, (k1 - k0) * 2 * KA]]),
                    Dhist[:, k0:k1, :, :].rearrange("p k l j -> p (k l j)"))
                nc.vector.memset(dums[:, c:c + 1], 1.0)

            # ---------- offsets (shared lane scale) ----------
            LN0 = sing.tile([BL, NEV], f32)
            nc.scalar.activation(LN0[:, 0:NEV - 1], MXs[:, 0:NEV - 1], AF.Ln)
            nc.vector.memset(LN0[:, NEV - 1:NEV], 0.0)
            onesb = sing.tile([BL, NEV], f32)
            nc.vector.memset(onesb[:], 1.0)
            OFA = sing.tile([BL, NEV + 1], f32)
            nc.vector.memset(OFA[:, 0:1], 0.0)
            nc.vector.tensor_tensor_scan(
                out=OFA[:, 1:NEV + 1], data0=onesb[:], data1=LN0[:],
                initial=0.0, op0=OP.mult, op1=OP.add)

            nc.sync.dma_start(
                bass.AP(off_d, 0, [[NEV + 1, BL], [1, NEV + 1]]), OFA[:])

            # ---------- gold via PE ----------
            patT = sing.tile([TC_, NPS], f32)
            nc.sync.dma_start(patT[0:KA, :], patR[:, :])

            gacc = sing.tile([128, BL, NPS], f32)
            nc.vector.memset(gacc[:], 0.0)

            for c in range(NGC):
                gps = psum.tile([128, BL * 16], f32, tag="gps")
                gpv = gps[:].rearrange("p (s c) -> p s c", c=16)
                ftc = gpool.tile([TC_, BL, KA], bf16, tag="ftc")
                nc.scalar.dma_start(ftc[:], featT[:, c, :, :])
                ohm = gpool.tile([TC_, BL, KA], bf16, tag="ohm")
                nc.scalar.dma_start(ohm[:], ohmT[:, c, :, :])
                ohp = gpool.tile([TC_, BL, KA + 1], bf16, tag="ohp")
                nc.scalar.dma_start(ohp[:], ohpT[:, c, :, :])

                for s in range(BL):
                    nc.tensor.matmul(
                        gpv[0:KA, s, 0:KA + 1],
                        lhsT=ohm[:, s, :], rhs=ohp[:, s, :],
                        start=True, stop=True, skip_group_check=True)
                    nc.tensor.matmul(
                        gpv[0:KA, s, KA + 1:NPS],
                        lhsT=ohm[:, s, :], rhs=ftc[:, s, :],
                        start=True, stop=True, skip_group_check=True)
                pc = min(c // 2, NCH - 2)
                nc.vector.scalar_tensor_tensor(
                    out=gacc[0:KA, :, :], in0=gpv[0:KA, :, 0:NPS],
                    scalar=dums[0:KA, pc:pc + 1], in1=gacc[0:KA, :, :],
                    op0=OP.mult, op1=OP.add)

            gj = spool.tile([128, BL, NPS], f32, tag="gj")
            nc.vector.tensor_tensor(
                gj[0:KA, :, :], gacc[0:KA, :, :],
                patT[0:KA, :].unsqueeze(1).broadcast_to([KA, BL, NPS]),
                op=OP.mult)
            gvecT = spool.tile([128, BL], f32, tag="gvecT")
            nc.vector.tensor_reduce(
                out=gvecT[0:KA, :], in_=gj[0:KA, :, :], axis=AX.X, op=OP.add)
            nc.sync.dma_start(
                bass.AP(gvec_d, 0, [[BL, KA], [1, BL]]), gvecT[0:KA, :])
            gvS = spool.tile([BL, KA], f32, tag="gvS")
            nc.sync.dma_start(gvS[:], bass.AP(gvec_d, 0, [[1, BL], [BL, KA]]))
            goldA = spool.tile([BL, 1], f32, tag="goldA")
            nc.vector.tensor_reduce(
                out=goldA[:], in_=gvS[:], axis=AX.X, op=OP.add)

            # ---------- last-tag term: trans[STOP, tag[len-1]] ----------
            tgl = spool.tile([BL, 1], bf16, tag="tgl")
            nc.gpsimd.indirect_dma_start(
                out=tgl[:], out_offset=None,
                in_=bass.AP(tagT, 0, [[1, TC_ * NGC * BL], [1, 1]]),
                in_offset=bass.IndirectOffsetOnAxis(ap=lidx_sb[:, 0:1], axis=0),
            )
            tglf = spool.tile([BL, 1], f32, tag="tglf")
            nc.vector.tensor_copy(tglf[:], tgl[:])
            ohl = spool.tile([BL, KA], f32, tag="ohl")
            nc.vector.scalar_tensor_tensor(
                out=ohl[:], in0=tglf[:].broadcast_to([BL, KA]), scalar=-1.0,
                in1=iota7[:], op0=OP.add, op1=OP.is_equal)
            jl = spool.tile([BL, KA], f32, tag="jl")
            nc.vector.tensor_tensor(
                jl[:], ohl[:], trv[:, STOP, 0:KA], op=OP.mult)
            lastp = spool.tile([BL, 1], f32, tag="lastp")
            nc.vector.tensor_reduce(out=lastp[:], in_=jl[:], axis=AX.X,
                                    op=OP.add)

            # ---------- extraction ----------
            Sg = spool.tile([BL, KA], bf16, tag="Sg")
            nc.gpsimd.indirect_dma_start(
                out=Sg[:], out_offset=None,
                in_=bass.AP(dh_d, 0, [[KA, BL * (H + 1) * 2], [1, KA]]),
                in_offset=bass.IndirectOffsetOnAxis(ap=hidx_sb[:, 0:1], axis=0),
            )
            offg = spool.tile([BL, 1], f32, tag="offg")
            nc.gpsimd.indirect_dma_start(
                out=offg[:], out_offset=None,
                in_=bass.AP(off_d, 0, [[1, 2 * BL * (NEV + 1)], [1, 1]]),
                in_offset=bass.IndirectOffsetOnAxis(ap=oidx_sb[:, 0:1], axis=0),
            )
            # partner = isl ? E @ alpha_m : E8   (beta hist stores d = efr*c)
            eaX = spool.tile([BL, KA, KA], bf16, tag="eaX")
            nc.vector.tensor_tensor(
                eaX[:], E2[:, 0, :, :],
                Dhist[:, H - 1, 0, :].unsqueeze(1).broadcast_to([BL, KA, KA]),
                op=OP.mult)
            amf = spool.tile([BL, KA], f32, tag="amf")
            nc.vector.tensor_reduce(out=amf[:], in_=eaX[:], axis=AX.X,
                                    op=OP.add)
            dmE = spool.tile([BL, KA], f32, tag="dmE")
            nc.vector.tensor_tensor(dmE[:], amf[:], E8E[:], op=OP.subtract)
            prt = spool.tile([BL, KA], f32, tag="prt")
            nc.vector.scalar_tensor_tensor(
                out=prt[:], in0=dmE[:], scalar=isl[:, 0:1], in1=E8E[:],
                op0=OP.mult, op1=OP.add)
            Sgf = spool.tile([BL, KA], f32, tag="Sgf")
            nc.vector.tensor_copy(Sgf[:], Sg[:])
            dotj = spool.tile([BL, KA], f32, tag="dotj")
            nc.vector.tensor_tensor(dotj[:], Sgf[:], prt[:], op=OP.mult)
            dots = spool.tile([BL, 1], f32, tag="dots")
            nc.vector.tensor_reduce(out=dots[:], in_=dotj[:], axis=AX.X,
                                    op=OP.add)
            nc.vector.tensor_scalar_max(dots[:], dots[:], 1e-38)
            lnv = spool.tile([BL, 1], f32, tag="lnv")
            nc.scalar.activation(lnv[:], dots[:], AF.Ln)
            # alpha_m (slot 1023) carries NEV-1 = 31 events -> OFF[31] lane 0
            at = spool.tile([BL, 1], f32, tag="at")
            nc.vector.scalar_tensor_tensor(
                out=at[:], in0=OFA[:, NEV - 1:NEV], scalar=isl[:, 0:1],
                in1=lnv[:], op0=OP.mult, op1=OP.add)
            f1 = spool.tile([BL, 1], f32, tag="f1")
            nc.vector.tensor_tensor(f1[:], at[:], offg[:], op=OP.add)
            fwdv = spool.tile([BL, 1], f32, tag="fwdv")
            nc.vector.scalar_tensor_tensor(
                out=fwdv[:], in0=lenf_sb[:], scalar=G, in1=f1[:],
                op0=OP.mult, op1=OP.add)

            # ---------- result ----------
            g2 = spool.tile([BL, 1], f32, tag="g2")
            nc.vector.tensor_tensor(g2[:], goldA[:], lastp[:], op=OP.add)
            res = spool.tile([BL, 1], f32, tag="res")
            nc.vector.tensor_tensor(res[:], fwdv[:], g2[:], op=OP.subtract)
            nc.sync.dma_start(outv[:, :], res[:])

    nc.finalize()
    return nc


def _prep_inputs(feats, transitions, tags, lengths):
    import ml_dtypes
    bf16 = ml_dtypes.bfloat16

    feats = np.asarray(feats, dtype=np.float32)
    transitions = np.asarray(transitions, dtype=np.float32)
    tags = np.asarray(tags).astype(np.int64)
    lengths = np.asarray(lengths).astype(np.int64)

    f7 = feats[:, :, :KA]
    fwd_f = np.ascontiguousarray(f7[:, :H]).astype(bf16)

    qi = np.arange(H)[None, :]
    src = lengths[:, None] - 1 - qi
    valid = src >= 0
    src_c = np.clip(src, 0, T - 1)
    gath = np.take_along_axis(f7, src_c[:, :, None], axis=1)
    rev_f = np.where(valid[:, :, None], gath, 0.0).astype(bf16)

    fT = np.ascontiguousarray(
        f7.reshape(B, NGC, TC_, KA).transpose(2, 1, 0, 3)).astype(bf16)
    tp1 = (tags + 1).astype(np.float32)
    tagT = np.ascontiguousarray(
        tp1.reshape(B, NGC, TC_).transpose(2, 1, 0)).astype(bf16)
    tprev = np.concatenate(
        [np.full((B, 1), START + 1, np.float32), tp1[:, :-1]], axis=1)
    mask = np.arange(T)[None, :] < lengths[:, None]
    mtag = np.where(mask, tp1, 0.0)
    iota7p = np.arange(1, KA + 1, dtype=np.float32)
    iota8p = np.arange(1, KA + 2, dtype=np.float32)
    ohm_f = (mtag[:, :, None] == iota7p[None, None, :])
    ohp_f = (tprev[:, :, None] == iota8p[None, None, :])
    ohmT = np.ascontiguousarray(
        ohm_f.reshape(B, NGC, TC_, KA).transpose(2, 1, 0, 3)).astype(bf16)
    ohpT = np.ascontiguousarray(
        ohp_f.reshape(B, NGC, TC_, KA + 1).transpose(2, 1, 0, 3)).astype(bf16)

    pat = np.zeros((KA, NPS), dtype=np.float32)
    pat[:, :KA + 1] = transitions[:KA, :KA + 1]
    pat[:, KA + 1:] = np.eye(KA, dtype=np.float32)

    # per-core index vectors
    sarr = np.arange(BL, dtype=np.int64)
    lm1 = lengths - 1
    is_long = (lm1 > H - 1)
    kq = np.where(is_long, lengths - H, lm1)            # q or t
    hidx = (sarr[None, :] * 0 + 0)  # placeholder; built per core below
    prep = {
        "fwd_f": fwd_f, "rev_f": rev_f, "featT": fT, "tagT": tagT,
        "ohmT": ohmT, "ohpT": ohpT,
        "lenf": lengths.astype(np.float32).reshape(B, 1),
        "trans": transitions, "patR": pat,
        "lm1": lm1, "is_long": is_long, "kq": kq,
    }
    return prep


def kernel(feats, transitions, tags, lengths):
    prep = _prep_inputs(feats, transitions, tags, lengths)

    if "nc" not in _CACHE:
        _CACHE["nc"] = _build_bass()
    nc = _CACHE["nc"]

    from concourse.bass_utils import run_bass_kernel_spmd

    lm1, is_long, kq = prep["lm1"], prep["is_long"], prep["kq"]
    sarr = np.arange(BL, dtype=np.int64)

    in_maps = []
    for c in range(NCORES):
        sl = slice(c * BL, (c + 1) * BL)
        lm1c, islc = lm1[sl], is_long[sl]
        kqc = np.where(islc, kq[sl] - 1, kq[sl])
        hidx = (sarr * (H + 1) * 2 + kqc * 2
                + np.where(islc, 1, 0)).astype(np.int32).reshape(BL, 1)
        evc = np.maximum(kqc - 4, 0) // RN
        oidx = (sarr * (NEV + 1) + evc).astype(np.int32).reshape(BL, 1)
        lidx = ((lm1c % TC_) * (NGC * BL) + (lm1c // TC_) * BL
                + sarr).astype(np.int32).reshape(BL, 1)
        in_maps.append({
            "fwd_f": prep["fwd_f"][sl], "rev_f": prep["rev_f"][sl],
            "featT": np.ascontiguousarray(prep["featT"][:, :, sl]),
            "tagT": np.ascontiguousarray(prep["tagT"][:, :, sl]),
            "ohmT": np.ascontiguousarray(prep["ohmT"][:, :, sl]),
            "ohpT": np.ascontiguousarray(prep["ohpT"][:, :, sl]),
            "lenf": prep["lenf"][sl],
            "hidx": hidx, "oidx": oidx, "lidx": lidx,
            "islf": islc.astype(np.float32).reshape(BL, 1),
            "trans": prep["trans"], "patR": prep["patR"],
        })
    r = run_bass_kernel_spmd(nc, in_maps, core_ids=list(range(NCORES)),
                             trace=TRACE)
    if TRACE:
        _CACHE["last_result"] = r
    per_seq = np.concatenate([m["outv"].reshape(BL) for m in r.results])
    _CACHE["per_seq"] = per_seq
    return np.float32(per_seq.mean(dtype=np.float64))


# revision 40
# speedup vs baseline: 1.0079x; 1.0079x over previous
"""BERT_BiLSTM_CRF loss (CRF NLL) Trainium2 kernel, v2.

Forward-backward meet-in-the-middle CRF forward scores + PE-based gold scores.

Per core (BL=128 seqs on partitions):
  - alpha chain (t=0..1023) and beta chain (q=0..1024) run as ONE dual-batched
    exp-space recurrence on DVE: state [BL,2,7], step matrices Mab [BL,2,7,7]
    built in bulk on Pool from ACT exp(feats-G). beta runs on host-reversed
    per-length zero-padded feats. fwd = ln(alpha_m . beta_q) + offsets for
    long seqs, ln(E8 . alpha_{len-1}) + offsets for short.
  - Renorm by running max every RN dual-steps (in-place on the stored history
    slot, per lane); ln(max) batched on ACT; prefix-sums via DVE scan.
  - History SBUF-resident, one flush to DRAM, per-seq indirect-DMA gathers
    with host-precomputed indices.
  - Gold via per-seq PE matmuls on host-transposed one-hot/feat planes:
    PSUM[j, s, 0:8]  = sum_t ohm[t,j] ohprev8[t,i]  (pair counts, START col)
    PSUM[j, s, 8:15] = sum_t ohm[t,j] featT[t,j']   (diagonal = feat part)
    dotted against [trans[0:7,0:8] | I7]; plus the STOP-row last-tag term.
Output: per-seq (fwd - gold) [BL,1] f32; host takes the mean.
"""

import os

import numpy as np

# Reset NeuronCores at runtime init: recovers degraded clock/p-state left by
# earlier runs (measured: identical NEFF 835us fresh vs 1000us degraded).
os.environ.setdefault("NEURON_RT_RESET_CORES", "1")

B, T, K = 1024, 2048, 9
NCORES = 8
BL = B // NCORES
KA = 7
START, STOP = 7, 8
G = 2.4
H = T // 2                 # chain length (dual steps)
RN = 32                    # dual-steps between renorms
NEV = H // RN              # renorm events per lane
CT = 128                   # dual-steps per M-build chunk
NCH = H // CT
TC_ = 128                  # t per transposed gold chunk
NGC = T // TC_             # gold chunks
NPS = 15                   # psum cols per seq (8 pair + 7 feat)

_CACHE = {}
TRACE = False


def _build_bass():
    import contextlib

    import concourse.bass as bass
    import concourse.bacc as bacc
    import concourse.mybir as mybir
    import concourse.tile as tile

    f32 = mybir.dt.float32
    bf16 = mybir.dt.bfloat16
    i32 = mybir.dt.int32
    AX = mybir.AxisListType
    OP = mybir.AluOpType
    AF = mybir.ActivationFunctionType

    nc = bacc.Bacc()

    # ---- DRAM inputs ----
    fwd_f = nc.dram_tensor("fwd_f", [BL, H, KA], bf16, kind="ExternalInput")
    rev_f = nc.dram_tensor("rev_f", [BL, H, KA], bf16, kind="ExternalInput")
    featT = nc.dram_tensor("featT", [TC_, NGC, BL, KA], bf16, kind="ExternalInput")
    tagT = nc.dram_tensor("tagT", [TC_, NGC, BL], bf16, kind="ExternalInput")
    ohmT = nc.dram_tensor("ohmT", [TC_, NGC, BL, KA], bf16, kind="ExternalInput")
    ohpT = nc.dram_tensor("ohpT", [TC_, NGC, BL, KA + 1], bf16,
                          kind="ExternalInput")
    lenf = nc.dram_tensor("lenf", [BL, 1], f32, kind="ExternalInput")
    hidx = nc.dram_tensor("hidx", [BL, 1], i32, kind="ExternalInput")
    oidx = nc.dram_tensor("oidx", [BL, 1], i32, kind="ExternalInput")
    lidx = nc.dram_tensor("lidx", [BL, 1], i32, kind="ExternalInput")
    islf = nc.dram_tensor("islf", [BL, 1], f32, kind="ExternalInput")
    trans = nc.dram_tensor("trans", [K, K], f32, kind="ExternalInput")
    patR = nc.dram_tensor("patR", [KA, NPS], f32, kind="ExternalInput")
    outv = nc.dram_tensor("outv", [BL, 1], f32, kind="ExternalOutput")

    # DRAM scratch
    dh_d = nc.dram_tensor("dh_d", [BL * (H + 1) * 2, KA], bf16)
    off_d = nc.dram_tensor("off_d", [2 * BL, NEV + 1], f32)
    gvec_d = nc.dram_tensor("gvec_d", [KA, BL], f32)

    iota7_np = np.arange(KA, dtype=np.float32).reshape(1, KA)
    c_iota7 = nc.inline_tensor(iota7_np, "c_iota7")
    iota8p1_np = np.arange(1, KA + 2, dtype=np.float32).reshape(1, KA + 1)
    c_iota8p1 = nc.inline_tensor(iota8p1_np, "c_iota8p1")

    with tile.TileContext(nc) as tc:
        ctx = contextlib.ExitStack()
        with ctx, nc.allow_low_precision(reason="bf16 CRF state, validated"):
            sing = ctx.enter_context(tc.tile_pool(name="sing", bufs=1))
            fpool = ctx.enter_context(tc.tile_pool(name="fpool", bufs=1))
            gpool = ctx.enter_context(tc.tile_pool(name="gpool", bufs=2))
            spool = ctx.enter_context(tc.tile_pool(name="spool", bufs=1))
            psum = ctx.enter_context(tc.tile_pool(name="psum", bufs=2,
                                                  space="PSUM"))

            # ---------- constants ----------
            transb = sing.tile([BL, K * K], f32)
            nc.sync.dma_start(transb[:], bass.AP(trans, 0, [[0, BL], [1, K * K]]))
            trv = transb[:].rearrange("p (j i) -> p j i", i=K)
            lenf_sb = sing.tile([BL, 1], f32)
            nc.sync.dma_start(lenf_sb[:], lenf[:, :])
            iota7 = sing.tile([BL, KA], f32)
            nc.sync.dma_start(iota7[:], bass.AP(c_iota7, 0, [[0, BL], [1, KA]]))
            hidx_sb = sing.tile([BL, 1], i32)
            nc.sync.dma_start(hidx_sb[:], hidx[:, :])
            oidx_sb = sing.tile([BL, 1], i32)
            nc.sync.dma_start(oidx_sb[:], oidx[:, :])
            lidx_sb = sing.tile([BL, 1], i32)
            nc.sync.dma_start(lidx_sb[:], lidx[:, :])
            isl = sing.tile([BL, 1], f32)
            nc.sync.dma_start(isl[:], islf[:, :])

            # E2[s, 0, j, i] = exp(trans[j, i]); E2[s, 1, a, b] = exp(trans[b, a])
            E2 = sing.tile([BL, 2, KA, KA], bf16)
            nc.scalar.activation(E2[:, 0, :, :], trv[:, 0:KA, 0:KA], AF.Exp)
            nc.scalar.activation(
                E2[:, 1, :, :],
                trv[:, 0:KA, 0:KA].rearrange("p j i -> p i j"), AF.Exp)
            E7E = sing.tile([BL, KA], bf16)      # exp(trans[j, START])
            nc.scalar.activation(E7E[:], trv[:, 0:KA, START:START + 1], AF.Exp)
            E8E = sing.tile([BL, KA], f32)       # exp(trans[STOP, j])
            nc.scalar.activation(E8E[:], trv[:, STOP:STOP + 1, 0:KA], AF.Exp)
            negG = sing.tile([BL, 1], f32)
            nc.vector.memset(negG[:], -G)

            # ---------- emissions (chunk-transient): efc local slot x holds
            # lane0 = exp(fwd[c*CT+x]-G) and lane1 = exp(rev[c*CT+x-1]-G),
            # x = 1..CT, exactly what M-build chunk c consumes ---------------
            ef00 = sing.tile([BL, 1, 2, KA], bf16)
            f0t = fpool.tile([BL, 1, KA], bf16, tag="fa")
            nc.sync.dma_start(f0t[:], fwd_f[:, 0:1, :])
            nc.scalar.activation(ef00[:, :, 0, :], f0t[:], AF.Exp,
                                 bias=negG[:, 0:1])
            f0r = fpool.tile([BL, 1, KA], bf16, tag="fb")
            nc.sync.dma_start(f0r[:], rev_f[:, 0:1, :])
            nc.scalar.activation(ef00[:, :, 1, :], f0r[:], AF.Exp,
                                 bias=negG[:, 0:1])

            # ---------- dual chain ----------
            Dhist = sing.tile([BL, H + 1, 2, KA], bf16)

            nc.vector.tensor_tensor(
                Dhist[:, 0, 0, :], E7E[:], ef00[:, 0, 0, :], op=OP.mult)
            nc.vector.tensor_tensor(
                Dhist[:, 0, 1, :], E8E[:], ef00[:, 0, 1, :], op=OP.mult)

            rcd = sing.tile([BL, 1], f32)
            MXs = sing.tile([BL, NEV], f32)
            dums = sing.tile([BL, NCH], f32)
            MT = [sing.tile([BL, CT, 2, KA, KA], bf16, name=f"MT{i}")
                  for i in range(6)]
            pend_at = [-1]
            for c in range(NCH):
                # fwd rows c*CT+1 .. c*CT+CT (last chunk: 127 rows + pad=1.0)
                nfw = CT if c < NCH - 1 else CT - 1
                efc = fpool.tile([BL, CT, 2, KA], bf16, tag="efc")
                fa = fpool.tile([BL, CT, KA], bf16, tag="fa")
                nc.sync.dma_start(fa[:, 0:nfw, :],
                                  fwd_f[:, c * CT + 1:c * CT + 1 + nfw, :])
                nc.scalar.activation(
                    efc[:, 0:nfw, 0, :], fa[:, 0:nfw, :],
                    AF.Exp, bias=negG[:, 0:1])
                if c == NCH - 1:
                    nc.vector.memset(efc[:, CT - 1, 0, :], 1.0)
                fb = fpool.tile([BL, CT, KA], bf16, tag="fb")
                nc.sync.dma_start(fb[:], rev_f[:, c * CT:(c + 1) * CT, :])
                nc.scalar.activation(
                    efc[:, :, 1, :], fb[:], AF.Exp, bias=negG[:, 0:1])

                Mab = MT[c % 6]
                ka0 = c * CT + 1
                nc.gpsimd.tensor_tensor(
                    Mab[:],
                    E2[:].unsqueeze(1).broadcast_to([BL, CT, 2, KA, KA]),
                    efc[:].unsqueeze(4).broadcast_to([BL, CT, 2, KA, KA]),
                    op=OP.mult)

                Xab = spool.tile([BL, 2, KA, KA], bf16, tag="Xab")
                for l in range(CT):
                    k = c * CT + l + 1
                    sprev = (Dhist[:, k - 1, :, :].unsqueeze(2)
                             .broadcast_to([BL, 2, KA, KA]))
                    if k == pend_at[0]:
                        for ch in range(2):
                            nc.vector.scalar_tensor_tensor(
                                out=Xab[:, ch, :, :],
                                in0=Mab[:, l, ch, :, :],
                                scalar=rcd[:, 0:1],
                                in1=Dhist[:, k - 1, ch, :].unsqueeze(1)
                                    .broadcast_to([BL, KA, KA]),
                                op0=OP.mult, op1=OP.mult)
                        pend_at[0] = -1
                    else:
                        nc.vector.tensor_tensor(
                            Xab[:], Mab[:, l, :, :, :], sprev, op=OP.mult)
                    nc.vector.tensor_reduce(
                        out=Dhist[:, k, :, :], in_=Xab[:], axis=AX.X, op=OP.add)
                    if k % RN == 0 and k < H:
                        ev = k // RN - 1
                        nc.vector.tensor_reduce(
                            out=MXs[:, ev:ev + 1], in_=Dhist[:, k, :, :],
                            axis=AX.XY, op=OP.max)
                        nc.vector.reciprocal(rcd[:], MXs[:, ev:ev + 1])
                        pend_at[0] = k + 4
                k0 = c * CT + 1 if c > 0 else 0
                k1 = (c + 1) * CT + 1
                nc.sync.dma_start(
                    bass.AP(dh_d, k0 * 2 * KA,
                            [[(H + 1) * 2 * KA, BL], [1, (k1 - k0) * 2 * KA]]),
                    Dhist[:, k0:k1, :, :].rearrange("p k l j -> p (k l j)"))
                nc.vector.memset(dums[:, c:c + 1], 1.0)

            # ---------- offsets (shared lane scale) ----------
            LN0 = sing.tile([BL, NEV], f32)
            nc.scalar.activation(LN0[:, 0:NEV - 1], MXs[:, 0:NEV - 1], AF.Ln)
            nc.vector.memset(LN0[:, NEV - 1:NEV], 0.0)
            onesb = sing.tile([BL, NEV], f32)
            nc.vector.memset(onesb[:], 1.0)
            OFA = sing.tile([BL, NEV + 1], f32)
            nc.vector.memset(OFA[:, 0:1], 0.0)
            nc.vector.tensor_tensor_scan(
                out=OFA[:, 1:NEV + 1], data0=onesb[:], data1=LN0[:],
                initial=0.0, op0=OP.mult, op1=OP.add)

            nc.sync.dma_start(
                bass.AP(off_d, 0, [[NEV + 1, BL], [1, NEV + 1]]), OFA[:])

            # ---------- gold via PE ----------
            patT = sing.tile([TC_, NPS], f32)
            nc.sync.dma_start(patT[0:KA, :], patR[:, :])

            gacc = sing.tile([128, BL, NPS], f32)
            nc.vector.memset(gacc[:], 0.0)

            for c in range(NGC):
                gps = psum.tile([128, BL * 16], f32, tag="gps")
                gpv = gps[:].rearrange("p (s c) -> p s c", c=16)
                ftc = gpool.tile([TC_, BL, KA], bf16, tag="ftc")
                nc.scalar.dma_start(ftc[:], featT[:, c, :, :])
                ohm = gpool.tile([TC_, BL, KA], bf16, tag="ohm")
                nc.scalar.dma_start(ohm[:], ohmT[:, c, :, :])
                ohp = gpool.tile([TC_, BL, KA + 1], bf16, tag="ohp")
                nc.scalar.dma_start(ohp[:], ohpT[:, c, :, :])

                for s in range(BL):
                    nc.tensor.matmul(
                        gpv[0:KA, s, 0:KA + 1],
                        lhsT=ohm[:, s, :], rhs=ohp[:, s, :],
                        start=True, stop=True, skip_group_check=True)
                    nc.tensor.matmul(
                        gpv[0:KA, s, KA + 1:NPS],
                        lhsT=ohm[:, s, :], rhs=ftc[:, s, :],
                        start=True, stop=True, skip_group_check=True)
                pc = min(c // 2, NCH - 2)
                nc.vector.scalar_tensor_tensor(
                    out=gacc[0:KA, :, :], in0=gpv[0:KA, :, 0:NPS],
                    scalar=dums[0:KA, pc:pc + 1], in1=gacc[0:KA, :, :],
                    op0=OP.mult, op1=OP.add)

            nc.vector.tensor_tensor(
                gacc[0:KA, :, :], gacc[0:KA, :, :],
                patT[0:KA, :].unsqueeze(1).broadcast_to([KA, BL, NPS]),
                op=OP.mult)
            gvecT = spool.tile([128, BL], f32, tag="gvecT")
            nc.vector.tensor_reduce(
                out=gvecT[0:KA, :], in_=gacc[0:KA, :, :], axis=AX.X, op=OP.add)
            nc.sync.dma_start(
                bass.AP(gvec_d, 0, [[BL, KA], [1, BL]]), gvecT[0:KA, :])
            gvS = spool.tile([BL, KA], f32, tag="gvS")
            nc.sync.dma_start(gvS[:], bass.AP(gvec_d, 0, [[1, BL], [BL, KA]]))
            goldA = spool.tile([BL, 1], f32, tag="goldA")
            nc.vector.tensor_reduce(
                out=goldA[:], in_=gvS[:], axis=AX.X, op=OP.add)

            # ---------- last-tag term: trans[STOP, tag[len-1]] ----------
            tgl = spool.tile([BL, 1], bf16, tag="tgl")
            nc.gpsimd.indirect_dma_start(
                out=tgl[:], out_offset=None,
                in_=bass.AP(tagT, 0, [[1, TC_ * NGC * BL], [1, 1]]),
                in_offset=bass.IndirectOffsetOnAxis(ap=lidx_sb[:, 0:1], axis=0),
            )
            tglf = spool.tile([BL, 1], f32, tag="tglf")
            nc.vector.tensor_copy(tglf[:], tgl[:])
            ohl = spool.tile([BL, KA], f32, tag="ohl")
            nc.vector.scalar_tensor_tensor(
                out=ohl[:], in0=tglf[:].broadcast_to([BL, KA]), scalar=-1.0,
                in1=iota7[:], op0=OP.add, op1=OP.is_equal)
            jl = spool.tile([BL, KA], f32, tag="jl")
            nc.vector.tensor_tensor(
                jl[:], ohl[:], trv[:, STOP, 0:KA], op=OP.mult)
            lastp = spool.tile([BL, 1], f32, tag="lastp")
            nc.vector.tensor_reduce(out=lastp[:], in_=jl[:], axis=AX.X,
                                    op=OP.add)

            # ---------- extraction ----------
            Sg = spool.tile([BL, KA], bf16, tag="Sg")
            nc.gpsimd.indirect_dma_start(
                out=Sg[:], out_offset=None,
                in_=bass.AP(dh_d, 0, [[KA, BL * (H + 1) * 2], [1, KA]]),
                in_offset=bass.IndirectOffsetOnAxis(ap=hidx_sb[:, 0:1], axis=0),
            )
            offg = spool.tile([BL, 1], f32, tag="offg")
            nc.gpsimd.indirect_dma_start(
                out=offg[:], out_offset=None,
                in_=bass.AP(off_d, 0, [[1, 2 * BL * (NEV + 1)], [1, 1]]),
                in_offset=bass.IndirectOffsetOnAxis(ap=oidx_sb[:, 0:1], axis=0),
            )
            # partner = isl ? E @ alpha_m : E8   (beta hist stores d = efr*c)
            eaX = spool.tile([BL, KA, KA], bf16, tag="eaX")
            nc.vector.tensor_tensor(
                eaX[:], E2[:, 0, :, :],
                Dhist[:, H - 1, 0, :].unsqueeze(1).broadcast_to([BL, KA, KA]),
                op=OP.mult)
            amf = spool.tile([BL, KA], f32, tag="amf")
            nc.vector.tensor_reduce(out=amf[:], in_=eaX[:], axis=AX.X,
                                    op=OP.add)
            dmE = spool.tile([BL, KA], f32, tag="dmE")
            nc.vector.tensor_tensor(dmE[:], amf[:], E8E[:], op=OP.subtract)
            prt = spool.tile([BL, KA], f32, tag="prt")
            nc.vector.scalar_tensor_tensor(
                out=prt[:], in0=dmE[:], scalar=isl[:, 0:1], in1=E8E[:],
                op0=OP.mult, op1=OP.add)
            Sgf = spool.tile([BL, KA], f32, tag="Sgf")
            nc.vector.tensor_copy(Sgf[:], Sg[:])
            dotj = spool.tile([BL, KA], f32, tag="dotj")
            nc.vector.tensor_tensor(dotj[:], Sgf[:], prt[:], op=OP.mult)
            dots = spool.tile([BL, 1], f32, tag="dots")
            nc.vector.tensor_reduce(out=dots[:], in_=dotj[:], axis=AX.X,
                                    op=OP.add)
            nc.vector.tensor_scalar_max(dots[:], dots[:], 1e-38)
            lnv = spool.tile([BL, 1], f32, tag="lnv")
            nc.scalar.activation(lnv[:], dots[:], AF.Ln)
            # alpha_m (slot 1023) carries NEV-1 = 31 events -> OFF[31] lane 0
            at = spool.tile([BL, 1], f32, tag="at")
            nc.vector.scalar_tensor_tensor(
                out=at[:], in0=OFA[:, NEV - 1:NEV], scalar=isl[:, 0:1],
                in1=lnv[:], op0=OP.mult, op1=OP.add)
            f1 = spool.tile([BL, 1], f32, tag="f1")
            nc.vector.tensor_tensor(f1[:], at[:], offg[:], op=OP.add)
            fwdv = spool.tile([BL, 1], f32, tag="fwdv")
            nc.vector.scalar_tensor_tensor(
                out=fwdv[:], in0=lenf_sb[:], scalar=G, in1=f1[:],
                op0=OP.mult, op1=OP.add)

            # ---------- result ----------
            g2 = spool.tile([BL, 1], f32, tag="g2")
            nc.vector.tensor_tensor(g2[:], goldA[:], lastp[:], op=OP.add)
            res = spool.tile([BL, 1], f32, tag="res")
            nc.vector.tensor_tensor(res[:], fwdv[:], g2[:], op=OP.subtract)
            nc.sync.dma_start(outv[:, :], res[:])

    nc.finalize()
    return nc


def _prep_inputs(feats, transitions, tags, lengths):
    import ml_dtypes
    bf16 = ml_dtypes.bfloat16

    feats = np.asarray(feats, dtype=np.float32)
    transitions = np.asarray(transitions, dtype=np.float32)
    tags = np.asarray(tags).astype(np.int64)
    lengths = np.asarray(lengths).astype(np.int64)

    f7 = feats[:, :, :KA]
    fwd_f = np.ascontiguousarray(f7[:, :H]).astype(bf16)

    qi = np.arange(H)[None, :]
    src = lengths[:, None] - 1 - qi
    valid = src >= 0
    src_c = np.clip(src, 0, T - 1)
    gath = np.take_along_axis(f7, src_c[:, :, None], axis=1)
    rev_f = np.where(valid[:, :, None], gath, 0.0).astype(bf16)

    fT = np.ascontiguousarray(
        f7.reshape(B, NGC, TC_, KA).transpose(2, 1, 0, 3)).astype(bf16)
    tp1 = (tags + 1).astype(np.float32)
    tagT = np.ascontiguousarray(
        tp1.reshape(B, NGC, TC_).transpose(2, 1, 0)).astype(bf16)
    tprev = np.concatenate(
        [np.full((B, 1), START + 1, np.float32), tp1[:, :-1]], axis=1)
    mask = np.arange(T)[None, :] < lengths[:, None]
    mtag = np.where(mask, tp1, 0.0)
    iota7p = np.arange(1, KA + 1, dtype=np.float32)
    iota8p = np.arange(1, KA + 2, dtype=np.float32)
    ohm_f = (mtag[:, :, None] == iota7p[None, None, :])
    ohp_f = (tprev[:, :, None] == iota8p[None, None, :])
    ohmT = np.ascontiguousarray(
        ohm_f.reshape(B, NGC, TC_, KA).transpose(2, 1, 0, 3)).astype(bf16)
    ohpT = np.ascontiguousarray(
        ohp_f.reshape(B, NGC, TC_, KA + 1).transpose(2, 1, 0, 3)).astype(bf16)

    pat = np.zeros((KA, NPS), dtype=np.float32)
    pat[:, :KA + 1] = transitions[:KA, :KA + 1]
    pat[:, KA + 1:] = np.eye(KA, dtype=np.float32)

    # per-core index vectors
    sarr = np.arange(BL, dtype=np.int64)
    lm1 = lengths - 1
    is_long = (lm1 > H - 1)
    kq = np.where(is_long, lengths - H, lm1)            # q or t
    hidx = (sarr[None, :] * 0 + 0)  # placeholder; built per core below
    prep = {
        "fwd_f": fwd_f, "rev_f": rev_f, "featT": fT, "tagT": tagT,
        "ohmT": ohmT, "ohpT": ohpT,
        "lenf": lengths.astype(np.float32).reshape(B, 1),
        "trans": transitions, "patR": pat,
        "lm1": lm1, "is_long": is_long, "kq": kq,
    }
    return prep


def kernel(feats, transitions, tags, lengths):
    prep = _prep_inputs(feats, transitions, tags, lengths)

    if "nc" not in _CACHE:
        _CACHE["nc"] = _build_bass()
    nc = _CACHE["nc"]

    from concourse.bass_utils import run_bass_kernel_spmd

    lm1, is_long, kq = prep["lm1"], prep["is_long"], prep["kq"]
    sarr = np.arange(BL, dtype=np.int64)

    in_maps = []
    for c in range(NCORES):
        sl = slice(c * BL, (c + 1) * BL)
        lm1c, islc = lm1[sl], is_long[sl]
        kqc = np.where(islc, kq[sl] - 1, kq[sl])
        hidx = (sarr * (H + 1) * 2 + kqc * 2
                + np.where(islc, 1, 0)).astype(np.int32).reshape(BL, 1)
        evc = np.maximum(kqc - 4, 0) // RN
        oidx = (sarr * (NEV + 1) + evc).astype(np.int32).reshape(BL, 1)
        lidx = ((lm1c % TC_) * (NGC * BL) + (lm1c // TC_) * BL
                + sarr).astype(np.int32).reshape(BL, 1)
        in_maps.append({
            "fwd_f": prep["fwd_f"][sl], "rev_f": prep["rev_f"][sl],
            "featT": np.ascontiguousarray(prep["featT"][:, :, sl]),
            "tagT": np.ascontiguousarray(prep["tagT"][:, :, sl]),
            "ohmT": np.ascontiguousarray(prep["ohmT"][:, :, sl]),
            "ohpT": np.ascontiguousarray(prep["ohpT"][:, :, sl]),
            "lenf": prep["lenf"][sl],
            "hidx": hidx, "oidx": oidx, "lidx": lidx,
            "islf": islc.astype(np.float32).reshape(BL, 1),
            "trans": prep["trans"], "patR": prep["patR"],
        })
    r = run_bass_kernel_spmd(nc, in_maps, core_ids=list(range(NCORES)),
                             trace=TRACE)
    if TRACE:
        _CACHE["last_result"] = r
    per_seq = np.concatenate([m["outv"].reshape(BL) for m in r.results])
    _CACHE["per_seq"] = per_seq
    return np.float32(per_seq.mean(dtype=np.float64))


# revision 41
# speedup vs baseline: 1.0106x; 1.0027x over previous
"""BERT_BiLSTM_CRF loss (CRF NLL) Trainium2 kernel, v2.

Forward-backward meet-in-the-middle CRF forward scores + PE-based gold scores.

Per core (BL=128 seqs on partitions):
  - alpha chain (t=0..1023) and beta chain (q=0..1024) run as ONE dual-batched
    exp-space recurrence on DVE: state [BL,2,7], step matrices Mab [BL,2,7,7]
    built in bulk on Pool from ACT exp(feats-G). beta runs on host-reversed
    per-length zero-padded feats. fwd = ln(alpha_m . beta_q) + offsets for
    long seqs, ln(E8 . alpha_{len-1}) + offsets for short.
  - Renorm by running max every RN dual-steps (in-place on the stored history
    slot, per lane); ln(max) batched on ACT; prefix-sums via DVE scan.
  - History SBUF-resident, one flush to DRAM, per-seq indirect-DMA gathers
    with host-precomputed indices.
  - Gold via per-seq PE matmuls on host-transposed one-hot/feat planes:
    PSUM[j, s, 0:8]  = sum_t ohm[t,j] ohprev8[t,i]  (pair counts, START col)
    PSUM[j, s, 8:15] = sum_t ohm[t,j] featT[t,j']   (diagonal = feat part)
    dotted against [trans[0:7,0:8] | I7]; plus the STOP-row last-tag term.
Output: per-seq (fwd - gold) [BL,1] f32; host takes the mean.
"""

import os

import numpy as np

# Reset NeuronCores at runtime init: recovers degraded clock/p-state left by
# earlier runs (measured: identical NEFF 835us fresh vs 1000us degraded).
os.environ.setdefault("NEURON_RT_RESET_CORES", "1")

B, T, K = 1024, 2048, 9
NCORES = 8
BL = B // NCORES
KA = 7
START, STOP = 7, 8
G = 2.4
H = T // 2                 # chain length (dual steps)
RN = 32                    # dual-steps between renorms
NEV = H // RN              # renorm events per lane
CT = 128                   # dual-steps per M-build chunk
NCH = H // CT
TC_ = 128                  # t per transposed gold chunk
NGC = T // TC_             # gold chunks
NPS = 15                   # psum cols per seq (8 pair + 7 feat)

_CACHE = {}
TRACE = False


def _build_bass():
    import contextlib

    import concourse.bass as bass
    import concourse.bacc as bacc
    import concourse.mybir as mybir
    import concourse.tile as tile

    f32 = mybir.dt.float32
    bf16 = mybir.dt.bfloat16
    i32 = mybir.dt.int32
    AX = mybir.AxisListType
    OP = mybir.AluOpType
    AF = mybir.ActivationFunctionType

    nc = bacc.Bacc()

    # ---- DRAM inputs ----
    fwd_f = nc.dram_tensor("fwd_f", [BL, H, KA], bf16, kind="ExternalInput")
    rev_f = nc.dram_tensor("rev_f", [BL, H, KA], bf16, kind="ExternalInput")
    featT = nc.dram_tensor("featT", [TC_, NGC, BL, KA], bf16, kind="ExternalInput")
    tagT = nc.dram_tensor("tagT", [TC_, NGC, BL], bf16, kind="ExternalInput")
    ohmT = nc.dram_tensor("ohmT", [TC_, NGC, BL, KA], bf16, kind="ExternalInput")
    ohpT = nc.dram_tensor("ohpT", [TC_, NGC, BL, KA + 1], bf16,
                          kind="ExternalInput")
    lenf = nc.dram_tensor("lenf", [BL, 1], f32, kind="ExternalInput")
    hidx = nc.dram_tensor("hidx", [BL, 1], i32, kind="ExternalInput")
    oidx = nc.dram_tensor("oidx", [BL, 1], i32, kind="ExternalInput")
    lidx = nc.dram_tensor("lidx", [BL, 1], i32, kind="ExternalInput")
    islf = nc.dram_tensor("islf", [BL, 1], f32, kind="ExternalInput")
    trans = nc.dram_tensor("trans", [K, K], f32, kind="ExternalInput")
    patR = nc.dram_tensor("patR", [KA, NPS], f32, kind="ExternalInput")
    outv = nc.dram_tensor("outv", [BL, 1], f32, kind="ExternalOutput")

    # DRAM scratch
    dh_d = nc.dram_tensor("dh_d", [BL * (H + 1) * 2, KA], bf16)
    off_d = nc.dram_tensor("off_d", [2 * BL, NEV + 1], f32)
    gvec_d = nc.dram_tensor("gvec_d", [KA, BL], f32)

    iota7_np = np.arange(KA, dtype=np.float32).reshape(1, KA)
    c_iota7 = nc.inline_tensor(iota7_np, "c_iota7")
    iota8p1_np = np.arange(1, KA + 2, dtype=np.float32).reshape(1, KA + 1)
    c_iota8p1 = nc.inline_tensor(iota8p1_np, "c_iota8p1")

    with tile.TileContext(nc) as tc:
        ctx = contextlib.ExitStack()
        with ctx, nc.allow_low_precision(reason="bf16 CRF state, validated"):
            sing = ctx.enter_context(tc.tile_pool(name="sing", bufs=1))
            fpool = ctx.enter_context(tc.tile_pool(name="fpool", bufs=1))
            gpool = ctx.enter_context(tc.tile_pool(name="gpool", bufs=2))
            spool = ctx.enter_context(tc.tile_pool(name="spool", bufs=1))
            psum = ctx.enter_context(tc.tile_pool(name="psum", bufs=2,
                                                  space="PSUM"))

            # ---------- constants ----------
            transb = sing.tile([BL, K * K], f32)
            nc.sync.dma_start(transb[:], bass.AP(trans, 0, [[0, BL], [1, K * K]]))
            trv = transb[:].rearrange("p (j i) -> p j i", i=K)
            lenf_sb = sing.tile([BL, 1], f32)
            nc.sync.dma_start(lenf_sb[:], lenf[:, :])
            iota7 = sing.tile([BL, KA], f32)
            nc.sync.dma_start(iota7[:], bass.AP(c_iota7, 0, [[0, BL], [1, KA]]))
            hidx_sb = sing.tile([BL, 1], i32)
            nc.sync.dma_start(hidx_sb[:], hidx[:, :])
            oidx_sb = sing.tile([BL, 1], i32)
            nc.sync.dma_start(oidx_sb[:], oidx[:, :])
            lidx_sb = sing.tile([BL, 1], i32)
            nc.sync.dma_start(lidx_sb[:], lidx[:, :])
            isl = sing.tile([BL, 1], f32)
            nc.sync.dma_start(isl[:], islf[:, :])

            # E2[s, 0, j, i] = exp(trans[j, i]); E2[s, 1, a, b] = exp(trans[b, a])
            E2 = sing.tile([BL, 2, KA, KA], bf16)
            nc.scalar.activation(E2[:, 0, :, :], trv[:, 0:KA, 0:KA], AF.Exp)
            nc.scalar.activation(
                E2[:, 1, :, :],
                trv[:, 0:KA, 0:KA].rearrange("p j i -> p i j"), AF.Exp)
            E7E = sing.tile([BL, KA], bf16)      # exp(trans[j, START])
            nc.scalar.activation(E7E[:], trv[:, 0:KA, START:START + 1], AF.Exp)
            E8E = sing.tile([BL, KA], f32)       # exp(trans[STOP, j])
            nc.scalar.activation(E8E[:], trv[:, STOP:STOP + 1, 0:KA], AF.Exp)
            negG = sing.tile([BL, 1], f32)
            nc.vector.memset(negG[:], -G)

            # ---------- emissions (chunk-transient): efc local slot x holds
            # lane0 = exp(fwd[c*CT+x]-G) and lane1 = exp(rev[c*CT+x-1]-G),
            # x = 1..CT, exactly what M-build chunk c consumes ---------------
            ef00 = sing.tile([BL, 1, 2, KA], bf16)
            f0t = fpool.tile([BL, 1, KA], bf16, tag="fa")
            nc.sync.dma_start(f0t[:], fwd_f[:, 0:1, :])
            nc.scalar.activation(ef00[:, :, 0, :], f0t[:], AF.Exp,
                                 bias=negG[:, 0:1])
            f0r = fpool.tile([BL, 1, KA], bf16, tag="fb")
            nc.sync.dma_start(f0r[:], rev_f[:, 0:1, :])
            nc.scalar.activation(ef00[:, :, 1, :], f0r[:], AF.Exp,
                                 bias=negG[:, 0:1])

            # ---------- dual chain ----------
            Dhist = sing.tile([BL, H + 1, 2, KA], bf16)

            nc.vector.tensor_tensor(
                Dhist[:, 0, 0, :], E7E[:], ef00[:, 0, 0, :], op=OP.mult)
            nc.vector.tensor_tensor(
                Dhist[:, 0, 1, :], E8E[:], ef00[:, 0, 1, :], op=OP.mult)

            rcd = sing.tile([BL, 1], f32)
            MXs = sing.tile([BL, NEV], f32)
            dums = sing.tile([BL, NCH], f32)
            MT = [sing.tile([BL, CT, 2, KA, KA], bf16, name=f"MT{i}")
                  for i in range(6)]
            pend_at = [-1]
            for c in range(NCH):
                # fwd rows c*CT+1 .. c*CT+CT (last chunk: 127 rows + pad=1.0)
                nfw = CT if c < NCH - 1 else CT - 1
                efc = fpool.tile([BL, CT, 2, KA], bf16, tag="efc")
                fa = fpool.tile([BL, CT, KA], bf16, tag="fa")
                nc.sync.dma_start(fa[:, 0:nfw, :],
                                  fwd_f[:, c * CT + 1:c * CT + 1 + nfw, :])
                nc.scalar.activation(
                    efc[:, 0:nfw, 0, :], fa[:, 0:nfw, :],
                    AF.Exp, bias=negG[:, 0:1])
                if c == NCH - 1:
                    nc.vector.memset(efc[:, CT - 1, 0, :], 1.0)
                fb = fpool.tile([BL, CT, KA], bf16, tag="fb")
                nc.sync.dma_start(fb[:], rev_f[:, c * CT:(c + 1) * CT, :])
                nc.scalar.activation(
                    efc[:, :, 1, :], fb[:], AF.Exp, bias=negG[:, 0:1])

                Mab = MT[c % 6]
                ka0 = c * CT + 1
                if c == 0:
                    # quarter-split so the chain can start after ~5us
                    QC = CT // 4
                    for qq in range(4):
                        nc.gpsimd.tensor_tensor(
                            Mab[:, qq * QC:(qq + 1) * QC],
                            E2[:].unsqueeze(1)
                                .broadcast_to([BL, QC, 2, KA, KA]),
                            efc[:, qq * QC:(qq + 1) * QC].unsqueeze(4)
                                .broadcast_to([BL, QC, 2, KA, KA]),
                            op=OP.mult)
                else:
                    nc.gpsimd.tensor_tensor(
                        Mab[:],
                        E2[:].unsqueeze(1).broadcast_to([BL, CT, 2, KA, KA]),
                        efc[:].unsqueeze(4).broadcast_to([BL, CT, 2, KA, KA]),
                        op=OP.mult)

                Xab = spool.tile([BL, 2, KA, KA], bf16, tag="Xab")
                for l in range(CT):
                    k = c * CT + l + 1
                    sprev = (Dhist[:, k - 1, :, :].unsqueeze(2)
                             .broadcast_to([BL, 2, KA, KA]))
                    if k == pend_at[0]:
                        for ch in range(2):
                            nc.vector.scalar_tensor_tensor(
                                out=Xab[:, ch, :, :],
                                in0=Mab[:, l, ch, :, :],
                                scalar=rcd[:, 0:1],
                                in1=Dhist[:, k - 1, ch, :].unsqueeze(1)
                                    .broadcast_to([BL, KA, KA]),
                                op0=OP.mult, op1=OP.mult)
                        pend_at[0] = -1
                    else:
                        nc.vector.tensor_tensor(
                            Xab[:], Mab[:, l, :, :, :], sprev, op=OP.mult)
                    nc.vector.tensor_reduce(
                        out=Dhist[:, k, :, :], in_=Xab[:], axis=AX.X, op=OP.add)
                    if k % RN == 0 and k < H:
                        ev = k // RN - 1
                        nc.vector.tensor_reduce(
                            out=MXs[:, ev:ev + 1], in_=Dhist[:, k, :, :],
                            axis=AX.XY, op=OP.max)
                        nc.vector.reciprocal(rcd[:], MXs[:, ev:ev + 1])
                        pend_at[0] = k + 4
                k0 = c * CT + 1 if c > 0 else 0
                k1 = (c + 1) * CT + 1
                nc.sync.dma_start(
                    bass.AP(dh_d, k0 * 2 * KA,
                            [[(H + 1) * 2 * KA, BL], [1, (k1 - k0) * 2 * KA]]),
                    Dhist[:, k0:k1, :, :].rearrange("p k l j -> p (k l j)"))
                nc.vector.memset(dums[:, c:c + 1], 1.0)

            # ---------- offsets (shared lane scale) ----------
            LN0 = sing.tile([BL, NEV], f32)
            nc.scalar.activation(LN0[:, 0:NEV - 1], MXs[:, 0:NEV - 1], AF.Ln)
            nc.vector.memset(LN0[:, NEV - 1:NEV], 0.0)
            onesb = sing.tile([BL, NEV], f32)
            nc.vector.memset(onesb[:], 1.0)
            OFA = sing.tile([BL, NEV + 1], f32)
            nc.vector.memset(OFA[:, 0:1], 0.0)
            nc.vector.tensor_tensor_scan(
                out=OFA[:, 1:NEV + 1], data0=onesb[:], data1=LN0[:],
                initial=0.0, op0=OP.mult, op1=OP.add)

            nc.sync.dma_start(
                bass.AP(off_d, 0, [[NEV + 1, BL], [1, NEV + 1]]), OFA[:])

            # ---------- gold via PE ----------
            patT = sing.tile([TC_, NPS], f32)
            nc.sync.dma_start(patT[0:KA, :], patR[:, :])

            gacc = sing.tile([128, BL, NPS], f32)
            nc.vector.memset(gacc[:], 0.0)

            for c in range(NGC):
                gps = psum.tile([128, BL * 16], f32, tag="gps")
                gpv = gps[:].rearrange("p (s c) -> p s c", c=16)
                ftc = gpool.tile([TC_, BL, KA], bf16, tag="ftc")
                nc.scalar.dma_start(ftc[:], featT[:, c, :, :])
                ohm = gpool.tile([TC_, BL, KA], bf16, tag="ohm")
                nc.scalar.dma_start(ohm[:], ohmT[:, c, :, :])
                ohp = gpool.tile([TC_, BL, KA + 1], bf16, tag="ohp")
                nc.scalar.dma_start(ohp[:], ohpT[:, c, :, :])

                for s in range(BL):
                    nc.tensor.matmul(
                        gpv[0:KA, s, 0:KA + 1],
                        lhsT=ohm[:, s, :], rhs=ohp[:, s, :],
                        start=True, stop=True, skip_group_check=True)
                    nc.tensor.matmul(
                        gpv[0:KA, s, KA + 1:NPS],
                        lhsT=ohm[:, s, :], rhs=ftc[:, s, :],
                        start=True, stop=True, skip_group_check=True)
                pc = min(c // 2, NCH - 2)
                nc.vector.scalar_tensor_tensor(
                    out=gacc[0:KA, :, :], in0=gpv[0:KA, :, 0:NPS],
                    scalar=dums[0:KA, pc:pc + 1], in1=gacc[0:KA, :, :],
                    op0=OP.mult, op1=OP.add)

            nc.vector.tensor_tensor(
                gacc[0:KA, :, :], gacc[0:KA, :, :],
                patT[0:KA, :].unsqueeze(1).broadcast_to([KA, BL, NPS]),
                op=OP.mult)
            gvecT = spool.tile([128, BL], f32, tag="gvecT")
            nc.vector.tensor_reduce(
                out=gvecT[0:KA, :], in_=gacc[0:KA, :, :], axis=AX.X, op=OP.add)
            nc.sync.dma_start(
                bass.AP(gvec_d, 0, [[BL, KA], [1, BL]]), gvecT[0:KA, :])
            gvS = spool.tile([BL, KA], f32, tag="gvS")
            nc.sync.dma_start(gvS[:], bass.AP(gvec_d, 0, [[1, BL], [BL, KA]]))
            goldA = spool.tile([BL, 1], f32, tag="goldA")
            nc.vector.tensor_reduce(
                out=goldA[:], in_=gvS[:], axis=AX.X, op=OP.add)

            # ---------- last-tag term: trans[STOP, tag[len-1]] ----------
            tgl = spool.tile([BL, 1], bf16, tag="tgl")
            nc.gpsimd.indirect_dma_start(
                out=tgl[:], out_offset=None,
                in_=bass.AP(tagT, 0, [[1, TC_ * NGC * BL], [1, 1]]),
                in_offset=bass.IndirectOffsetOnAxis(ap=lidx_sb[:, 0:1], axis=0),
            )
            tglf = spool.tile([BL, 1], f32, tag="tglf")
            nc.vector.tensor_copy(tglf[:], tgl[:])
            ohl = spool.tile([BL, KA], f32, tag="ohl")
            nc.vector.scalar_tensor_tensor(
                out=ohl[:], in0=tglf[:].broadcast_to([BL, KA]), scalar=-1.0,
                in1=iota7[:], op0=OP.add, op1=OP.is_equal)
            jl = spool.tile([BL, KA], f32, tag="jl")
            nc.vector.tensor_tensor(
                jl[:], ohl[:], trv[:, STOP, 0:KA], op=OP.mult)
            lastp = spool.tile([BL, 1], f32, tag="lastp")
            nc.vector.tensor_reduce(out=lastp[:], in_=jl[:], axis=AX.X,
                                    op=OP.add)

            # ---------- extraction ----------
            Sg = spool.tile([BL, KA], bf16, tag="Sg")
            nc.gpsimd.indirect_dma_start(
                out=Sg[:], out_offset=None,
                in_=bass.AP(dh_d, 0, [[KA, BL * (H + 1) * 2], [1, KA]]),
                in_offset=bass.IndirectOffsetOnAxis(ap=hidx_sb[:, 0:1], axis=0),
            )
            offg = spool.tile([BL, 1], f32, tag="offg")
            nc.gpsimd.indirect_dma_start(
                out=offg[:], out_offset=None,
                in_=bass.AP(off_d, 0, [[1, 2 * BL * (NEV + 1)], [1, 1]]),
                in_offset=bass.IndirectOffsetOnAxis(ap=oidx_sb[:, 0:1], axis=0),
            )
            # partner = isl ? E @ alpha_m : E8   (beta hist stores d = efr*c)
            eaX = spool.tile([BL, KA, KA], bf16, tag="eaX")
            nc.vector.tensor_tensor(
                eaX[:], E2[:, 0, :, :],
                Dhist[:, H - 1, 0, :].unsqueeze(1).broadcast_to([BL, KA, KA]),
                op=OP.mult)
            amf = spool.tile([BL, KA], f32, tag="amf")
            nc.vector.tensor_reduce(out=amf[:], in_=eaX[:], axis=AX.X,
                                    op=OP.add)
            dmE = spool.tile([BL, KA], f32, tag="dmE")
            nc.vector.tensor_tensor(dmE[:], amf[:], E8E[:], op=OP.subtract)
            prt = spool.tile([BL, KA], f32, tag="prt")
            nc.vector.scalar_tensor_tensor(
                out=prt[:], in0=dmE[:], scalar=isl[:, 0:1], in1=E8E[:],
                op0=OP.mult, op1=OP.add)
            Sgf = spool.tile([BL, KA], f32, tag="Sgf")
            nc.vector.tensor_copy(Sgf[:], Sg[:])
            dotj = spool.tile([BL, KA], f32, tag="dotj")
            nc.vector.tensor_tensor(dotj[:], Sgf[:], prt[:], op=OP.mult)
            dots = spool.tile([BL, 1], f32, tag="dots")
            nc.vector.tensor_reduce(out=dots[:], in_=dotj[:], axis=AX.X,
                                    op=OP.add)
            nc.vector.tensor_scalar_max(dots[:], dots[:], 1e-38)
            lnv = spool.tile([BL, 1], f32, tag="lnv")
            nc.scalar.activation(lnv[:], dots[:], AF.Ln)
            # alpha_m (slot 1023) carries NEV-1 = 31 events -> OFF[31] lane 0
            at = spool.tile([BL, 1], f32, tag="at")
            nc.vector.scalar_tensor_tensor(
                out=at[:], in0=OFA[:, NEV - 1:NEV], scalar=isl[:, 0:1],
                in1=lnv[:], op0=OP.mult, op1=OP.add)
            f1 = spool.tile([BL, 1], f32, tag="f1")
            nc.vector.tensor_tensor(f1[:], at[:], offg[:], op=OP.add)
            fwdv = spool.tile([BL, 1], f32, tag="fwdv")
            nc.vector.scalar_tensor_tensor(
                out=fwdv[:], in0=lenf_sb[:], scalar=G, in1=f1[:],
                op0=OP.mult, op1=OP.add)

            # ---------- result ----------
            g2 = spool.tile([BL, 1], f32, tag="g2")
            nc.vector.tensor_tensor(g2[:], goldA[:], lastp[:], op=OP.add)
            res = spool.tile([BL, 1], f32, tag="res")
            nc.vector.tensor_tensor(res[:], fwdv[:], g2[:], op=OP.subtract)
            nc.sync.dma_start(outv[:, :], res[:])

    nc.finalize()
    return nc


def _prep_inputs(feats, transitions, tags, lengths):
    import ml_dtypes
    bf16 = ml_dtypes.bfloat16

    feats = np.asarray(feats, dtype=np.float32)
    transitions = np.asarray(transitions, dtype=np.float32)
    tags = np.asarray(tags).astype(np.int64)
    lengths = np.asarray(lengths).astype(np.int64)

    f7 = feats[:, :, :KA]
    fwd_f = np.ascontiguousarray(f7[:, :H]).astype(bf16)

    qi = np.arange(H)[None, :]
    src = lengths[:, None] - 1 - qi
    valid = src >= 0
    src_c = np.clip(src, 0, T - 1)
    gath = np.take_along_axis(f7, src_c[:, :, None], axis=1)
    rev_f = np.where(valid[:, :, None], gath, 0.0).astype(bf16)

    fT = np.ascontiguousarray(
        f7.reshape(B, NGC, TC_, KA).transpose(2, 1, 0, 3)).astype(bf16)
    tp1 = (tags + 1).astype(np.float32)
    tagT = np.ascontiguousarray(
        tp1.reshape(B, NGC, TC_).transpose(2, 1, 0)).astype(bf16)
    tprev = np.concatenate(
        [np.full((B, 1), START + 1, np.float32), tp1[:, :-1]], axis=1)
    mask = np.arange(T)[None, :] < lengths[:, None]
    mtag = np.where(mask, tp1, 0.0)
    iota7p = np.arange(1, KA + 1, dtype=np.float32)
    iota8p = np.arange(1, KA + 2, dtype=np.float32)
    ohm_f = (mtag[:, :, None] == iota7p[None, None, :])
    ohp_f = (tprev[:, :, None] == iota8p[None, None, :])
    ohmT = np.ascontiguousarray(
        ohm_f.reshape(B, NGC, TC_, KA).transpose(2, 1, 0, 3)).astype(bf16)
    ohpT = np.ascontiguousarray(
        ohp_f.reshape(B, NGC, TC_, KA + 1).transpose(2, 1, 0, 3)).astype(bf16)

    pat = np.zeros((KA, NPS), dtype=np.float32)
    pat[:, :KA + 1] = transitions[:KA, :KA + 1]
    pat[:, KA + 1:] = np.eye(KA, dtype=np.float32)

    # per-core index vectors
    sarr = np.arange(BL, dtype=np.int64)
    lm1 = lengths - 1
    is_long = (lm1 > H - 1)
    kq = np.where(is_long, lengths - H, lm1)            # q or t
    hidx = (sarr[None, :] * 0 + 0)  # placeholder; built per core below
    prep = {
        "fwd_f": fwd_f, "rev_f": rev_f, "featT": fT, "tagT": tagT,
        "ohmT": ohmT, "ohpT": ohpT,
        "lenf": lengths.astype(np.float32).reshape(B, 1),
        "trans": transitions, "patR": pat,
        "lm1": lm1, "is_long": is_long, "kq": kq,
    }
    return prep


def kernel(feats, transitions, tags, lengths):
    prep = _prep_inputs(feats, transitions, tags, lengths)

    if "nc" not in _CACHE:
        _CACHE["nc"] = _build_bass()
    nc = _CACHE["nc"]

    from concourse.bass_utils import run_bass_kernel_spmd

    lm1, is_long, kq = prep["lm1"], prep["is_long"], prep["kq"]
    sarr = np.arange(BL, dtype=np.int64)

    in_maps = []
    for c in range(NCORES):
        sl = slice(c * BL, (c + 1) * BL)
        lm1c, islc = lm1[sl], is_long[sl]
        kqc = np.where(islc, kq[sl] - 1, kq[sl])
        hidx = (sarr * (H + 1) * 2 + kqc * 2
                + np.where(islc, 1, 0)).astype(np.int32).reshape(BL, 1)
        evc = np.maximum(kqc - 4, 0) // RN
        oidx = (sarr * (NEV + 1) + evc).astype(np.int32).reshape(BL, 1)
        lidx = ((lm1c % TC_) * (NGC * BL) + (lm1c // TC_) * BL
                + sarr).astype(np.int32).reshape(BL, 1)
        in_maps.append({
            "fwd_f": prep["fwd_f"][sl], "rev_f": prep["rev_f"][sl],
            "featT": np.ascontiguousarray(prep["featT"][:, :, sl]),
            "tagT": np.ascontiguousarray(prep["tagT"][:, :, sl]),
            "ohmT": np.ascontiguousarray(prep["ohmT"][:, :, sl]),
            "ohpT": np.ascontiguousarray(prep["ohpT"][:, :, sl]),
            "lenf": prep["lenf"][sl],
            "hidx": hidx, "oidx": oidx, "lidx": lidx,
            "islf": islc.astype(np.float32).reshape(BL, 1),
            "trans": prep["trans"], "patR": prep["patR"],
        })
    r = run_bass_kernel_spmd(nc, in_maps, core_ids=list(range(NCORES)),
                             trace=TRACE)
    if TRACE:
        _CACHE["last_result"] = r
    per_seq = np.concatenate([m["outv"].reshape(BL) for m in r.results])
    _CACHE["per_seq"] = per_seq
    return np.float32(per_seq.mean(dtype=np.float64))
